# revision 4
# baseline (speedup 1.0000x reference)
"""Causal single-head attention (N=4096, D=F=1024) on 8 TRN2 NeuronCores.

Sequence-parallel sharding: core c owns query rows [512c, 512(c+1)).

Two SPMD launches:
  A) QKV projection — each core computes q/k/v for its own 512 rows
     (weights replicated, host pre-transposed to contraction-major layouts).
  B) attention + output projection — each core consumes its q.T plus
     full k.T / v that the host has right-aligned ("rotated") so that the
     causal diagonal always lands in the LAST 512-column block regardless
     of core id. Junk key columns are zeroed (their exp(0)=1 probabilities
     multiply zeroed v rows and a zeroed ones-column, so they contribute
     nothing); the remaining triangular mask is core-invariant and applied
     on-chip with affine_select. This keeps the SPMD program fully uniform
     across cores with no control flow.

Matmuls run as float32r (full PE rate at free-dim 512, ~fp32 accuracy).
"""

import sys

try:
    import concourse.bass as bass
except ImportError:  # pragma: no cover
    sys.path.insert(0, "/opt/trn_rl_repo")
    import concourse.bass as bass

import numpy as np

import concourse.mybir as mybir
import concourse.tile as tile
from concourse import bacc
from concourse.bass_utils import run_bass_kernel_spmd

N, D, F = 4096, 1024, 1024
C = 8              # cores
NL = N // C        # 512 query rows per core
P = 128
SCALE = 1.0 / float(np.sqrt(np.float32(F)))

F32 = mybir.dt.float32
MM_DT = mybir.dt.float32r  # matmul compute dtype (bitcast view of f32 data)

DT = D // P        # 8 contraction tiles
FT = F // P        # 8 f tiles
MT = N // P        # 32 key tiles
NT2 = NL // P      # 4 query-row tiles per core

# Filled with [launchA_ns, launchB_ns] when BASS_TRACE=1 profiling is active.
LAST_EXEC_NS = [None, None]
LAST_RESULTS = [None, None]

_CACHE = {}


def _mm(x):
    return x


def _build_qkv():
    nc = bacc.Bacc(None, target_bir_lowering=False)
    xT = nc.dram_tensor("xT", [P, DT, NL], MM_DT, kind="ExternalInput")
    wqT = nc.dram_tensor("wqT", [D, F], MM_DT, kind="ExternalInput")
    wkT = nc.dram_tensor("wkT", [D, F], MM_DT, kind="ExternalInput")
    wvT = nc.dram_tensor("wvT", [D, F], MM_DT, kind="ExternalInput")
    bq = nc.dram_tensor("bq", [P, FT], F32, kind="ExternalInput")
    bk = nc.dram_tensor("bk", [P, FT], F32, kind="ExternalInput")
    bvB = nc.dram_tensor("bvB", [P, F], F32, kind="ExternalInput")
    qT_o = nc.dram_tensor("qT_o", [F, NL], F32, kind="ExternalOutput")
    kT_o = nc.dram_tensor("kT_o", [F, NL], F32, kind="ExternalOutput")
    v_o = nc.dram_tensor("v_o", [NL, F], F32, kind="ExternalOutput")

    with tile.TileContext(nc) as tc:
        with (
            tc.tile_pool(name="singles", bufs=1) as singles,
            tc.tile_pool(name="weights", bufs=1) as weights,
            tc.tile_pool(name="osb", bufs=3) as opool,
            tc.tile_pool(name="psum", bufs=4, space="PSUM") as psum,
        ):
            xT_sb = singles.tile([P, DT, NL], MM_DT)
            nc.sync.dma_start(out=xT_sb, in_=xT.ap())
            bq_sb = singles.tile([P, FT], F32)
            nc.sync.dma_start(out=bq_sb, in_=bq.ap())
            bk_sb = singles.tile([P, FT], F32)
            nc.sync.dma_start(out=bk_sb, in_=bk.ap())
            bvB_sb = singles.tile([P, F], F32)
            nc.sync.dma_start(out=bvB_sb, in_=bvB.ap())

            wq_sb = weights.tile([P, DT, F], MM_DT, tag="wq")
            nc.sync.dma_start(
                out=wq_sb, in_=wqT.ap().rearrange("(t p) f -> p t f", p=P)
            )
            wk_sb = weights.tile([P, DT, F], MM_DT, tag="wk")
            nc.sync.dma_start(
                out=wk_sb, in_=wkT.ap().rearrange("(t p) f -> p t f", p=P)
            )
            wv_sb = weights.tile([P, DT, F], MM_DT, tag="wv")
            nc.sync.dma_start(
                out=wv_sb, in_=wvT.ap().rearrange("(t p) f -> p t f", p=P)
            )

            # q.T / k.T : out[f_tile, n] = sum_d wT[d, f] * xT[d, n]
            for w_sb, b_sb, out_t in ((wq_sb, bq_sb, qT_o), (wk_sb, bk_sb, kT_o)):
                for ft in range(FT):
                    ps = psum.tile([P, NL], F32, tag="ps")
                    for dt_i in range(DT):
                        nc.tensor.matmul(
                            ps,
                            _mm(w_sb[:, dt_i, ft * P : (ft + 1) * P]),
                            _mm(xT_sb[:, dt_i, :]),
                            start=(dt_i == 0),
                            stop=(dt_i == DT - 1),
                        )
                    osb = opool.tile([P, NL], F32, tag="osb")
                    nc.scalar.activation(
                        out=osb,
                        in_=ps,
                        func=mybir.ActivationFunctionType.Identity,
                        bias=b_sb[:, ft : ft + 1],
                        scale=1.0,
                    )
                    nc.sync.dma_start(out=out_t.ap()[ft * P : (ft + 1) * P, :], in_=osb)

            # v : out[m_tile, f] = sum_d xT[d, m] * wvT[d, f]
            for fc in range(2):
                fs = slice(fc * 512, (fc + 1) * 512)
                for mi in range(NT2):
                    ps = psum.tile([P, 512], F32, tag="ps")
                    for dt_i in range(DT):
                        nc.tensor.matmul(
                            ps,
                            _mm(xT_sb[:, dt_i, mi * P : (mi + 1) * P]),
                            _mm(wv_sb[:, dt_i, fs]),
                            start=(dt_i == 0),
                            stop=(dt_i == DT - 1),
                        )
                    vsb = opool.tile([P, 512], F32, tag="osb")
                    nc.vector.tensor_add(out=vsb, in0=ps, in1=bvB_sb[:, fs])
                    nc.sync.dma_start(
                        out=v_o.ap()[mi * P : (mi + 1) * P, fs], in_=vsb
                    )
    nc.finalize()
    return nc


def _build_attn():
    nc = bacc.Bacc(None, target_bir_lowering=False)
    qT = nc.dram_tensor("qT", [P, FT, NL], MM_DT, kind="ExternalInput")
    kTr = nc.dram_tensor("kTr", [F, N], MM_DT, kind="ExternalInput")
    vblk = nc.dram_tensor("vblk", [FT, P, MT, P], MM_DT, kind="ExternalInput")
    ones = nc.dram_tensor("ones", [P, MT], MM_DT, kind="ExternalInput")
    projT = nc.dram_tensor("projT", [F, F], MM_DT, kind="ExternalInput")
    pbB = nc.dram_tensor("pbB", [P, F], F32, kind="ExternalInput")
    out_o = nc.dram_tensor("out_o", [NL, F], F32, kind="ExternalOutput")

    with tile.TileContext(nc) as tc:
        with (
            tc.tile_pool(name="singles", bufs=1) as singles,
            tc.tile_pool(name="kc", bufs=2) as kpool,
            tc.tile_pool(name="pt", bufs=MT) as ptpool,
            tc.tile_pool(name="vc", bufs=3) as vpool,
            tc.tile_pool(name="osb", bufs=3) as opool,
            tc.tile_pool(name="sps", bufs=2, space="PSUM") as spsum,
            tc.tile_pool(name="rps", bufs=1, space="PSUM") as rpsum,
            tc.tile_pool(name="zps", bufs=2, space="PSUM") as zpsum,
            tc.tile_pool(name="ops", bufs=2, space="PSUM") as opsum,
            tc.tile_pool(name="dram", bufs=1, space="DRAM") as drampool,
        ):
            qT_sb = singles.tile([P, FT, NL], MM_DT)
            nc.sync.dma_start(out=qT_sb, in_=qT.ap())
            ones_sb = singles.tile([P, MT], MM_DT)
            nc.sync.dma_start(out=ones_sb, in_=ones.ap())
            pbB_sb = singles.tile([P, F], F32)
            nc.sync.dma_start(out=pbB_sb, in_=pbB.ap())
            projT_sb = singles.tile([P, FT, F], MM_DT)
            nc.sync.dma_start(
                out=projT_sb, in_=projT.ap().rearrange("(t p) f -> p t f", p=P)
            )
            z_sb = singles.tile([P, FT, NL], MM_DT)

            # ---- scores + exp:  pT[m, n] = exp(SCALE * sum_f kTr[f, m] qT[f, n])
            pts = []
            for mc in range(MT // 2):  # key chunks of 256 columns
                kc = kpool.tile([P, FT, 2 * P], MM_DT, tag="kc")
                nc.sync.dma_start(
                    out=kc,
                    in_=kTr.ap()[:, mc * 2 * P : (mc + 1) * 2 * P].rearrange(
                        "(t p) m -> p t m", p=P
                    ),
                )
                for mi in range(2):
                    mt = 2 * mc + mi
                    ps = spsum.tile([P, NL], F32, tag="sps")
                    for ft in range(FT):
                        nc.tensor.matmul(
                            ps,
                            _mm(kc[:, ft, mi * P : (mi + 1) * P]),
                            _mm(qT_sb[:, ft, :]),
                            start=(ft == 0),
                            stop=(ft == FT - 1),
                        )
                    pt = ptpool.tile([P, NL], MM_DT, tag="pt")
                    nc.scalar.activation(
                        out=pt,
                        in_=ps,
                        func=mybir.ActivationFunctionType.Exp,
                        scale=SCALE,
                    )
                    if mt >= MT - NT2:
                        # diagonal block: keep only m_local <= n
                        nc.gpsimd.affine_select(
                            out=pt,
                            in_=pt,
                            pattern=[[1, NL]],
                            compare_op=mybir.AluOpType.is_ge,
                            fill=0.0,
                            base=-(mt - (MT - NT2)) * P,
                            channel_multiplier=-1,
                        )
                    pts.append(pt)

            # ---- row sums (over valid keys only) via ones-column matmuls
            rps = rpsum.tile([1, NL], F32)
            for mt in range(MT):
                nc.tensor.matmul(
                    rps,
                    _mm(ones_sb[:, mt : mt + 1]),
                    _mm(pts[mt]),
                    start=(mt == 0),
                    stop=(mt == MT - 1),
                )
            recip_row = singles.tile([1, NL], F32)
            nc.vector.reciprocal(out=recip_row, in_=rps)
            scratch = drampool.tile([1, NL], F32)
            nc.sync.dma_start(out=scratch, in_=recip_row)
            recip_np = singles.tile([P, NT2], F32)
            nc.sync.dma_start(
                out=recip_np, in_=scratch[0].rearrange("(t p) -> p t", p=P)
            )

            # ---- z.T[f, n] = sum_m v[m, f] * pT[m, n]
            for ft in range(FT):
                for vh in range(2):  # half-chunks of 16 key tiles
                    vc = vpool.tile([P, MT // 2, P], MM_DT, tag="vc")
                    nc.sync.dma_start(
                        out=vc, in_=vblk.ap()[ft, :, vh * 16 : (vh + 1) * 16, :]
                    )
                    if vh == 0:
                        zps = zpsum.tile([P, NL], F32, tag="zps")
                    for mi in range(MT // 2):
                        mt = vh * 16 + mi
                        nc.tensor.matmul(
                            zps,
                            _mm(vc[:, mi, :]),
                            _mm(pts[mt]),
                            start=(mt == 0),
                            stop=(mt == MT - 1),
                        )
                nc.vector.tensor_copy(out=z_sb[:, ft, :], in_=zps)

            # ---- out[n, o] = (z.T/rowsum) @ projT + pb
            for nt in range(NT2):
                for oc in range(2):
                    os_ = slice(oc * 512, (oc + 1) * 512)
                    ops = opsum.tile([P, 512], F32, tag="ops")
                    for ft in range(FT):
                        nc.tensor.matmul(
                            ops,
                            _mm(z_sb[:, ft, nt * P : (nt + 1) * P]),
                            _mm(projT_sb[:, ft, os_]),
                            start=(ft == 0),
                            stop=(ft == FT - 1),
                        )
                    osb = opool.tile([P, 512], F32, tag="osb")
                    nc.vector.scalar_tensor_tensor(
                        out=osb,
                        in0=ops,
                        scalar=recip_np[:, nt : nt + 1],
                        in1=pbB_sb[:, os_],
                        op0=mybir.AluOpType.mult,
                        op1=mybir.AluOpType.add,
                    )
                    nc.sync.dma_start(
                        out=out_o.ap()[nt * P : (nt + 1) * P, os_], in_=osb
                    )
    nc.finalize()
    return nc


def _get_programs():
    if "qkv" not in _CACHE:
        _CACHE["qkv"] = _build_qkv()
        _CACHE["attn"] = _build_attn()
    return _CACHE["qkv"], _CACHE["attn"]


def _c(a):
    return np.ascontiguousarray(a, dtype=np.float32)


def kernel(x, wq_w, wq_b, wk_w, wk_b, wv_w, wv_b, proj_w, proj_b):
    x = np.asarray(x, dtype=np.float32)
    nc_qkv, nc_attn = _get_programs()

    # ---- launch A: QKV projection, sequence-sharded
    xT = _c(np.asarray(x).T)                      # [D, N]
    wqT = _c(np.asarray(wq_w).T)                  # [D, F]
    wkT = _c(np.asarray(wk_w).T)
    wvT = _c(np.asarray(wv_w).T)
    bq_pb = _c(np.asarray(wq_b).reshape(FT, P).T)   # [P, FT]
    bk_pb = _c(np.asarray(wk_b).reshape(FT, P).T)
    bvB = _c(np.broadcast_to(np.asarray(wv_b), (P, F)))
    in_a = []
    for c in range(C):
        xT_blk = _c(
            xT[:, c * NL : (c + 1) * NL].reshape(DT, P, NL).transpose(1, 0, 2)
        )
        in_a.append(
            {
                "xT": xT_blk,
                "wqT": wqT,
                "wkT": wkT,
                "wvT": wvT,
                "bq": bq_pb,
                "bk": bk_pb,
                "bvB": bvB,
            }
        )
    res_a = run_bass_kernel_spmd(nc_qkv, in_a, core_ids=list(range(C)))
    LAST_EXEC_NS[0] = res_a.exec_time_ns
    LAST_RESULTS[0] = res_a

    kT_full = np.concatenate([res_a.results[c]["kT_o"] for c in range(C)], axis=1)
    v_full = np.concatenate([res_a.results[c]["v_o"] for c in range(C)], axis=0)

    # ---- launch B: attention + projection
    projT = _c(np.asarray(proj_w).T)              # [F, F]
    pbB = _c(np.broadcast_to(np.asarray(proj_b), (P, F)))
    in_b = []
    for c in range(C):
        L = NL * (c + 1)          # valid key rows for this core
        J = N - L                 # zero-padded junk columns (multiple of 512)
        qT_blk = _c(res_a.results[c]["qT_o"].reshape(FT, P, NL).transpose(1, 0, 2))
        kTr = np.zeros((F, N), dtype=np.float32)
        kTr[:, J:] = kT_full[:, :L]
        vblk = np.zeros((FT, P, MT, P), dtype=np.float32)
        # valid rows occupy whole key tiles: t >= J // P
        vblk[:, :, J // P :, :] = (
            v_full[:L].reshape(L // P, P, FT, P).transpose(2, 1, 0, 3)
        )
        ones_pb = np.zeros((P, MT), dtype=np.float32)
        ones_pb[:, J // P :] = 1.0
        in_b.append(
            {
                "qT": qT_blk,
                "kTr": kTr,
                "vblk": vblk,
                "ones": ones_pb,
                "projT": projT,
                "pbB": pbB,
            }
        )
    res_b = run_bass_kernel_spmd(nc_attn, in_b, core_ids=list(range(C)))
    LAST_EXEC_NS[1] = res_b.exec_time_ns
    LAST_RESULTS[1] = res_b

    return np.concatenate([res_b.results[c]["out_o"] for c in range(C)], axis=0)


# revision 5
# speedup vs baseline: 1.1657x; 1.1657x over previous
"""Causal single-head attention (N=4096, D=F=1024) on 8 TRN2 NeuronCores.

Sequence-parallel sharding: core c owns query rows [512c, 512(c+1)).

Two SPMD launches:
  A) QKV projection — each core computes q/k/v for its own 512 rows
     (weights replicated, host pre-transposed to contraction-major layouts).
  B) attention + output projection — each core consumes its q.T plus
     full k.T / v that the host has right-aligned ("rotated") so that the
     causal diagonal always lands in the LAST 512-column block regardless
     of core id. Junk key columns are zeroed (their exp(0)=1 probabilities
     multiply zeroed v rows and a zeroed ones-column, so they contribute
     nothing); the remaining triangular mask is core-invariant and applied
     on-chip with affine_select. This keeps the SPMD program fully uniform
     across cores with no control flow.

Matmuls run as float32r (full PE rate at free-dim 512, ~fp32 accuracy).
"""

import sys

try:
    import concourse.bass as bass
except ImportError:  # pragma: no cover
    sys.path.insert(0, "/opt/trn_rl_repo")
    import concourse.bass as bass

import numpy as np

import concourse.mybir as mybir
import concourse.tile as tile
from concourse import bacc
from concourse.bass_utils import run_bass_kernel_spmd

N, D, F = 4096, 1024, 1024
C = 8              # cores
NL = N // C        # 512 query rows per core
P = 128
SCALE = 1.0 / float(np.sqrt(np.float32(F)))

F32 = mybir.dt.float32
MM_DT = mybir.dt.float32r  # matmul compute dtype (bitcast view of f32 data)

DT = D // P        # 8 contraction tiles
FT = F // P        # 8 f tiles
MT = N // P        # 32 key tiles
NT2 = NL // P      # 4 query-row tiles per core

# Filled with [launchA_ns, launchB_ns] when BASS_TRACE=1 profiling is active.
LAST_EXEC_NS = [None, None]
LAST_RESULTS = [None, None]

_CACHE = {}


def _mm(x):
    return x


def _build_qkv():
    nc = bacc.Bacc(None, target_bir_lowering=False)
    xT = nc.dram_tensor("xT", [P, DT, NL], MM_DT, kind="ExternalInput")
    wqT = nc.dram_tensor("wqT", [D, F], MM_DT, kind="ExternalInput")
    wkT = nc.dram_tensor("wkT", [D, F], MM_DT, kind="ExternalInput")
    wvT = nc.dram_tensor("wvT", [D, F], MM_DT, kind="ExternalInput")
    bq = nc.dram_tensor("bq", [P, FT], F32, kind="ExternalInput")
    bk = nc.dram_tensor("bk", [P, FT], F32, kind="ExternalInput")
    bvB = nc.dram_tensor("bvB", [P, F], F32, kind="ExternalInput")
    qT_o = nc.dram_tensor("qT_o", [F, NL], F32, kind="ExternalOutput")
    kT_o = nc.dram_tensor("kT_o", [F, NL], F32, kind="ExternalOutput")
    v_o = nc.dram_tensor("v_o", [NL, F], F32, kind="ExternalOutput")

    with tile.TileContext(nc) as tc:
        with (
            tc.tile_pool(name="singles", bufs=1) as singles,
            tc.tile_pool(name="weights", bufs=4) as weights,
            tc.tile_pool(name="osb", bufs=3) as opool,
            tc.tile_pool(name="psum", bufs=4, space="PSUM") as psum,
        ):
            xT_sb = singles.tile([P, DT, NL], MM_DT)
            nc.sync.dma_start(out=xT_sb, in_=xT.ap())
            bq_sb = singles.tile([P, FT], F32)
            nc.sync.dma_start(out=bq_sb, in_=bq.ap())
            bk_sb = singles.tile([P, FT], F32)
            nc.sync.dma_start(out=bk_sb, in_=bk.ap())
            bvB_sb = singles.tile([P, F], F32)
            nc.sync.dma_start(out=bvB_sb, in_=bvB.ap())

            # q.T / k.T : out[f_tile, n] = sum_d wT[d, f] * xT[d, n]
            # weights streamed in per-f-tile chunks so PE starts early
            for w_t, b_sb, out_t in ((wqT, bq_sb, qT_o), (wkT, bk_sb, kT_o)):
                for ft in range(FT):
                    wc = weights.tile([P, DT, P], MM_DT, tag="wc")
                    nc.sync.dma_start(
                        out=wc,
                        in_=w_t.ap()[:, ft * P : (ft + 1) * P].rearrange(
                            "(t p) f -> p t f", p=P
                        ),
                    )
                    ps = psum.tile([P, NL], F32, tag="ps")
                    for dt_i in range(DT):
                        nc.tensor.matmul(
                            ps,
                            _mm(wc[:, dt_i, :]),
                            _mm(xT_sb[:, dt_i, :]),
                            start=(dt_i == 0),
                            stop=(dt_i == DT - 1),
                        )
                    osb = opool.tile([P, NL], F32, tag="osb")
                    nc.vector.tensor_scalar_add(
                        out=osb, in0=ps, scalar1=b_sb[:, ft : ft + 1]
                    )
                    nc.sync.dma_start(out=out_t.ap()[ft * P : (ft + 1) * P, :], in_=osb)

            # v : out[m_tile, f] = sum_d xT[d, m] * wvT[d, f]
            for fc in range(2):
                fs = slice(fc * 512, (fc + 1) * 512)
                wvc = weights.tile([P, DT, 512], MM_DT, tag="wvc")
                nc.sync.dma_start(
                    out=wvc,
                    in_=wvT.ap()[:, fs].rearrange("(t p) f -> p t f", p=P),
                )
                for mi in range(NT2):
                    ps = psum.tile([P, 512], F32, tag="ps")
                    for dt_i in range(DT):
                        nc.tensor.matmul(
                            ps,
                            _mm(xT_sb[:, dt_i, mi * P : (mi + 1) * P]),
                            _mm(wvc[:, dt_i, :]),
                            start=(dt_i == 0),
                            stop=(dt_i == DT - 1),
                        )
                    vsb = opool.tile([P, 512], F32, tag="osb")
                    nc.vector.tensor_add(out=vsb, in0=ps, in1=bvB_sb[:, fs])
                    nc.sync.dma_start(
                        out=v_o.ap()[mi * P : (mi + 1) * P, fs], in_=vsb
                    )
    nc.finalize()
    return nc


def _build_attn():
    nc = bacc.Bacc(None, target_bir_lowering=False)
    qT = nc.dram_tensor("qT", [P, FT, NL], MM_DT, kind="ExternalInput")
    kTr = nc.dram_tensor("kTr", [F, N], MM_DT, kind="ExternalInput")
    vblk = nc.dram_tensor("vblk", [FT, P, MT, P], MM_DT, kind="ExternalInput")
    ones = nc.dram_tensor("ones", [P, MT], MM_DT, kind="ExternalInput")
    projT = nc.dram_tensor("projT", [F, F], MM_DT, kind="ExternalInput")
    pbB = nc.dram_tensor("pbB", [P, F], F32, kind="ExternalInput")
    out_o = nc.dram_tensor("out_o", [NL, F], F32, kind="ExternalOutput")

    with tile.TileContext(nc) as tc:
        with (
            tc.tile_pool(name="singles", bufs=1) as singles,
            tc.tile_pool(name="kc", bufs=2) as kpool,
            tc.tile_pool(name="pt", bufs=MT) as ptpool,
            tc.tile_pool(name="vc", bufs=3) as vpool,
            tc.tile_pool(name="osb", bufs=3) as opool,
            tc.tile_pool(name="sps", bufs=2, space="PSUM") as spsum,
            tc.tile_pool(name="rps", bufs=1, space="PSUM") as rpsum,
            tc.tile_pool(name="zps", bufs=2, space="PSUM") as zpsum,
            tc.tile_pool(name="ops", bufs=2, space="PSUM") as opsum,
            tc.tile_pool(name="dram", bufs=1, space="DRAM") as drampool,
        ):
            qT_sb = singles.tile([P, FT, NL], MM_DT)
            nc.sync.dma_start(out=qT_sb, in_=qT.ap())
            ones_sb = singles.tile([P, MT], MM_DT)
            nc.sync.dma_start(out=ones_sb, in_=ones.ap())
            z_sb = singles.tile([P, FT, NL], MM_DT)

            # ---- scores + exp:  pT[m, n] = exp(SCALE * sum_f kTr[f, m] qT[f, n])
            pts = []
            for mc in range(MT // 4):  # key chunks of 512 columns
                kc = kpool.tile([P, FT, 4 * P], MM_DT, tag="kc")
                nc.sync.dma_start(
                    out=kc,
                    in_=kTr.ap()[:, mc * 4 * P : (mc + 1) * 4 * P].rearrange(
                        "(t p) m -> p t m", p=P
                    ),
                )
                for mi in range(4):
                    mt = 4 * mc + mi
                    ps = spsum.tile([P, NL], F32, tag="sps")
                    for ft in range(FT):
                        nc.tensor.matmul(
                            ps,
                            _mm(kc[:, ft, mi * P : (mi + 1) * P]),
                            _mm(qT_sb[:, ft, :]),
                            start=(ft == 0),
                            stop=(ft == FT - 1),
                        )
                    pt = ptpool.tile([P, NL], MM_DT, tag="pt")
                    nc.scalar.activation(
                        out=pt,
                        in_=ps,
                        func=mybir.ActivationFunctionType.Exp,
                        scale=SCALE,
                    )
                    if mt >= MT - NT2:
                        # diagonal block: keep only m_local <= n
                        nc.gpsimd.affine_select(
                            out=pt,
                            in_=pt,
                            pattern=[[1, NL]],
                            compare_op=mybir.AluOpType.is_ge,
                            fill=0.0,
                            base=-(mt - (MT - NT2)) * P,
                            channel_multiplier=-1,
                        )
                    pts.append(pt)

            # ---- row sums (over valid keys only) via ones-column matmuls
            rps = rpsum.tile([1, NL], F32)
            for mt in range(MT):
                nc.tensor.matmul(
                    rps,
                    _mm(ones_sb[:, mt : mt + 1]),
                    _mm(pts[mt]),
                    start=(mt == 0),
                    stop=(mt == MT - 1),
                )
            recip_row = singles.tile([1, NL], F32)
            nc.vector.reciprocal(out=recip_row, in_=rps)
            scratch = drampool.tile([1, NL], F32)
            nc.sync.dma_start(out=scratch, in_=recip_row)
            recip_np = singles.tile([P, NT2], F32)
            nc.sync.dma_start(
                out=recip_np, in_=scratch[0].rearrange("(t p) -> p t", p=P)
            )

            # ---- z.T[f, n] = sum_m v[m, f] * pT[m, n]
            for ft in range(FT):
                for vh in range(2):  # half-chunks of 16 key tiles
                    vc = vpool.tile([P, MT // 2, P], MM_DT, tag="vc")
                    nc.sync.dma_start(
                        out=vc, in_=vblk.ap()[ft, :, vh * 16 : (vh + 1) * 16, :]
                    )
                    if vh == 0:
                        zps = zpsum.tile([P, NL], F32, tag="zps")
                    for mi in range(MT // 2):
                        mt = vh * 16 + mi
                        nc.tensor.matmul(
                            zps,
                            _mm(vc[:, mi, :]),
                            _mm(pts[mt]),
                            start=(mt == 0),
                            stop=(mt == MT - 1),
                        )
                nc.vector.tensor_copy(out=z_sb[:, ft, :], in_=zps)

            # ---- out[n, o] = (z.T/rowsum) @ projT + pb
            pbB_sb = singles.tile([P, F], F32)
            nc.sync.dma_start(out=pbB_sb, in_=pbB.ap())
            projT_sb = singles.tile([P, FT, F], MM_DT)
            nc.sync.dma_start(
                out=projT_sb, in_=projT.ap().rearrange("(t p) f -> p t f", p=P)
            )
            for nt in range(NT2):
                for oc in range(2):
                    os_ = slice(oc * 512, (oc + 1) * 512)
                    ops = opsum.tile([P, 512], F32, tag="ops")
                    for ft in range(FT):
                        nc.tensor.matmul(
                            ops,
                            _mm(z_sb[:, ft, nt * P : (nt + 1) * P]),
                            _mm(projT_sb[:, ft, os_]),
                            start=(ft == 0),
                            stop=(ft == FT - 1),
                        )
                    osb = opool.tile([P, 512], F32, tag="osb")
                    nc.vector.scalar_tensor_tensor(
                        out=osb,
                        in0=ops,
                        scalar=recip_np[:, nt : nt + 1],
                        in1=pbB_sb[:, os_],
                        op0=mybir.AluOpType.mult,
                        op1=mybir.AluOpType.add,
                    )
                    nc.sync.dma_start(
                        out=out_o.ap()[nt * P : (nt + 1) * P, os_], in_=osb
                    )
    nc.finalize()
    return nc


def _get_programs():
    if "qkv" not in _CACHE:
        _CACHE["qkv"] = _build_qkv()
        _CACHE["attn"] = _build_attn()
    return _CACHE["qkv"], _CACHE["attn"]


def _c(a):
    return np.ascontiguousarray(a, dtype=np.float32)


def kernel(x, wq_w, wq_b, wk_w, wk_b, wv_w, wv_b, proj_w, proj_b):
    x = np.asarray(x, dtype=np.float32)
    nc_qkv, nc_attn = _get_programs()

    # ---- launch A: QKV projection, sequence-sharded
    xT = _c(np.asarray(x).T)                      # [D, N]
    wqT = _c(np.asarray(wq_w).T)                  # [D, F]
    wkT = _c(np.asarray(wk_w).T)
    wvT = _c(np.asarray(wv_w).T)
    bq_pb = _c(np.asarray(wq_b).reshape(FT, P).T)   # [P, FT]
    bk_pb = _c(np.asarray(wk_b).reshape(FT, P).T)
    bvB = _c(np.broadcast_to(np.asarray(wv_b), (P, F)))
    in_a = []
    for c in range(C):
        xT_blk = _c(
            xT[:, c * NL : (c + 1) * NL].reshape(DT, P, NL).transpose(1, 0, 2)
        )
        in_a.append(
            {
                "xT": xT_blk,
                "wqT": wqT,
                "wkT": wkT,
                "wvT": wvT,
                "bq": bq_pb,
                "bk": bk_pb,
                "bvB": bvB,
            }
        )
    res_a = run_bass_kernel_spmd(nc_qkv, in_a, core_ids=list(range(C)))
    LAST_EXEC_NS[0] = res_a.exec_time_ns
    LAST_RESULTS[0] = res_a

    kT_full = np.concatenate([res_a.results[c]["kT_o"] for c in range(C)], axis=1)
    v_full = np.concatenate([res_a.results[c]["v_o"] for c in range(C)], axis=0)

    # ---- launch B: attention + projection
    projT = _c(np.asarray(proj_w).T)              # [F, F]
    pbB = _c(np.broadcast_to(np.asarray(proj_b), (P, F)))
    in_b = []
    for c in range(C):
        L = NL * (c + 1)          # valid key rows for this core
        J = N - L                 # zero-padded junk columns (multiple of 512)
        qT_blk = _c(res_a.results[c]["qT_o"].reshape(FT, P, NL).transpose(1, 0, 2))
        kTr = np.zeros((F, N), dtype=np.float32)
        kTr[:, J:] = kT_full[:, :L]
        vblk = np.zeros((FT, P, MT, P), dtype=np.float32)
        # valid rows occupy whole key tiles: t >= J // P
        vblk[:, :, J // P :, :] = (
            v_full[:L].reshape(L // P, P, FT, P).transpose(2, 1, 0, 3)
        )
        ones_pb = np.zeros((P, MT), dtype=np.float32)
        ones_pb[:, J // P :] = 1.0
        in_b.append(
            {
                "qT": qT_blk,
                "kTr": kTr,
                "vblk": vblk,
                "ones": ones_pb,
                "projT": projT,
                "pbB": pbB,
            }
        )
    res_b = run_bass_kernel_spmd(nc_attn, in_b, core_ids=list(range(C)))
    LAST_EXEC_NS[1] = res_b.exec_time_ns
    LAST_RESULTS[1] = res_b

    return np.concatenate([res_b.results[c]["out_o"] for c in range(C)], axis=0)
